# revision 7
# baseline (speedup 1.0000x reference)
"""DGT block (dynamic graph transformer) Bass kernel for Trainium2 — v2.

Sharding: 8 cores = 4 batches x 2 query-halves (2048 queries/core vs all
4096 keys).

Key structure (vs v1):
  - kNN scores via ONE fp32r matmul per 512-key chunk with rows
    [32*f_n | bias-row B=2^24+2^22]. The PSUM write rounds (guard bit,
    single binade) to round(32*f.f) + B exactly.
  - One DVE scalar_tensor_tensor pass fuses PSUM eviction, de-bias, the
    norm subtraction and index packing:  u = (t - B) + C  where
    C = iota - round(16*||f||^2)  (device-built in setup).  u's integer
    part orders by quantized score (step 1/32), its fraction encodes the
    key index:  frac = (idx+0.25) * 2^-12.
  - top-16 = max8 per 512-window + match_replace merge; indices come from
    the fraction arithmetically (NO max_index full-row scans).
  - gather rows [xT | vT | pos,1] (768B); k/v projections fold into the
    attention matmuls post-gather (matmul cost is output-bound).
  - ww = v_g + pe, uu = ee*ww at bf16 2x; softmax reduces as tree adds
    (level 1 on DVE at 2x, levels 2-4 on Pool).
"""

import numpy as np
import ml_dtypes

B, N, K, DP, DM, EPS = 4, 4096, 16, 64, 128, 1e-5
NQ = N // 2            # queries per core
TQ = 128               # queries per tile
NT = NQ // TQ          # tiles per core (16)
PAIR = TQ * K          # pairs per tile (2048)
ROW = 3 * DM           # table row elems (f16): [xT(128) | vT(128) | pos4 | pad]
BIAS = float(2 ** 24 + 2 ** 22)
CC = float(2 ** 23 + 2 ** 22)

_CACHE = {}

bf16 = ml_dtypes.bfloat16


def _fold_bn(p):
    g, be, m, v = p.astype(np.float64)
    s = g / np.sqrt(v + EPS)
    return s.astype(np.float32), (be - m * s).astype(np.float32)


def _build_bass():
    import concourse.bass as bass
    import concourse.mybir as mybir
    import concourse.bacc as bacc
    from concourse.tile import TileContext

    dt = mybir.dt
    AF = mybir.ActivationFunctionType
    ALU = mybir.AluOpType

    nc = bacc.Bacc("TRN2", target_bir_lowering=False, debug=False, num_devices=8)

    def inp(name, shape, dtype):
        return nc.dram_tensor(name, list(shape), dtype, kind="ExternalInput").ap()

    lhsr_d = inp("lhsr", (65, NQ), dt.float32r)     # [32*f_own | ones]
    rhsr_d = inp("rhsr", (65, N), dt.float32r)      # [f_all | B-row]
    fb_res_d = inp("fb_res", (DP, NQ), dt.float32)  # residual
    feats32_d = inp("feats32", (DP, N), dt.float32)
    featsbf_d = inp("featsbf", (DP, N), dt.bfloat16)
    pos_own_d = inp("pos_own", (3, NQ), dt.bfloat16)
    fob_d = inp("fob", (DP, NQ), dt.bfloat16)
    posT_d = inp("posT", (N, 4), dt.float16)
    iota_d = inp("iota_in", (128, N), dt.float32)
    normL_d = inp("normL", (65, 128), dt.float32)   # [-16 x64 | 1]
    w1t_d = inp("W1fT", (DP, DM), dt.bfloat16)
    b1_d = inp("b1", (DM, 1), dt.float32)
    wvT_d = inp("WvT", (DM, DM), dt.bfloat16)
    identbf_d = inp("identbf", (DM, DM), dt.bfloat16)
    identf_d = inp("identf", (TQ, TQ), dt.float32)
    wd1fT_d = inp("Wd1fT", (3, DM), dt.bfloat16)
    wd1augT_d = inp("Wd1augT", (4, DM), dt.bfloat16)   # [-Wd1f^T ; bd1]
    wd2t_d = inp("Wd2fT", (DM, DM), dt.bfloat16)
    bd2_d = inp("bd2", (DM, 1), dt.float32)
    wq1t_d = inp("Wg1qT", (DM, DM), dt.bfloat16)
    wg1kt_d = inp("Wg1kTneg", (DM, DM), dt.bfloat16)   # -(Wg1f Wk)^T
    wg1t_d = inp("Wg1fT", (DM, DM), dt.bfloat16)
    bg1_d = inp("bg1", (DM, 1), dt.float32)
    wg2t_d = inp("Wg2fT", (DM, DM), dt.bfloat16)
    bg2_d = inp("bg2", (DM, 1), dt.float32)
    w2t_d = inp("W2fT", (DM, DP), dt.bfloat16)
    b2_d = inp("b2", (DP, 1), dt.float32)
    e_d = inp("E", (TQ, PAIR), dt.bfloat16)

    out_d = nc.dram_tensor("out", [DP, NQ], dt.float32, kind="ExternalOutput").ap()

    f32, f16, bft, i16 = dt.float32, dt.float16, dt.bfloat16, dt.int16

    with TileContext(nc) as tc:
        with (
            tc.tile_pool(name="const", bufs=1) as cpool,
            tc.tile_pool(name="persist", bufs=1) as ppool,
            tc.tile_pool(name="dram", bufs=1, space="DRAM") as dpool,
            tc.tile_pool(name="sc", bufs=2, space="PSUM") as scps,
            tc.tile_pool(name="zp", bufs=2, space="PSUM") as zpps,
        ):
            # ---- persistent constants ----
            lhsr = cpool.tile_from(lhsr_d)
            rhsr = cpool.tile_from(rhsr_d)
            w1t = cpool.tile_from(w1t_d)
            b1 = cpool.tile_from(b1_d)
            wvT = cpool.tile_from(wvT_d)
            identbf = cpool.tile_from(identbf_d)
            identf = cpool.tile_from(identf_d)
            wd1fT = cpool.tile_from(wd1fT_d)
            wd1augT = cpool.tile_from(wd1augT_d)
            wd2t = cpool.tile_from(wd2t_d)
            bd2 = cpool.tile_from(bd2_d)
            wq1t = cpool.tile_from(wq1t_d)
            wg1kt = cpool.tile_from(wg1kt_d)
            wg1t = cpool.tile_from(wg1t_d)
            bg1 = cpool.tile_from(bg1_d)
            wg2t = cpool.tile_from(wg2t_d)
            bg2 = cpool.tile_from(bg2_d)
            w2t = cpool.tile_from(w2t_d)
            b2 = cpool.tile_from(b2_d)
            emat = cpool.tile_from(e_d)
            normL = cpool.tile_from(normL_d)

            # ---- persistent working tensors ----
            C = ppool.tile([128, N], f32)          # iota - round(16||f||^2)
            xfull = ppool.tile([DM, N], bft)
            gqT = ppool.tile([TQ, NQ], bft)
            gpT = ppool.tile([TQ, NQ], bft)
            res_all = ppool.tile([DM, NQ], bft)
            fb_res = ppool.tile_from(fb_res_d)
            table = dpool.tile([N, ROW], f16)

            # ================= Phase A: setup =================
            with (
                tc.tile_pool(name="setup", bufs=2) as spool,
                tc.tile_pool(name="setup1", bufs=1) as s1pool,
            ):
                feats32 = s1pool.tile_from(feats32_d)
                featsbf = s1pool.tile_from(featsbf_d)
                poso = s1pool.tile_from(pos_own_d)
                fob = s1pool.tile_from(fob_d)
                iota = s1pool.tile_from(iota_d)

                # pos -> table cols [256:260]
                nc.sync.dma_start(out=table[:, 2 * DM:2 * DM + 4], in_=posT_d)

                # C = iota - round(16||f||^2): ffB rows [f*f | B]
                ffB = s1pool.tile([65, N], f32)
                nc.vector.memset(ffB[64:65, :], BIAS)
                for s in range(8):
                    nc.gpsimd.tensor_mul(ffB[0:64, bass.ts(s, 512)],
                                         feats32[:, bass.ts(s, 512)],
                                         feats32[:, bass.ts(s, 512)])
                for s in range(8):
                    ps = scps.tile([128, 512], f32, tag="sc")
                    nc.tensor.matmul(ps[:], normL[:], ffB[:, bass.ts(s, 512)],
                                     start=True, stop=True)
                    nc.vector.scalar_tensor_tensor(
                        out=C[:, bass.ts(s, 512)], in0=ps[:], scalar=BIAS,
                        in1=iota[:, bass.ts(s, 512)], op0=ALU.subtract, op1=ALU.add)

                # x = lrelu(bn1(W1 f)) for all keys
                for s in range(8):
                    ps = zpps.tile([DM, 1024], f32, tag="zp")
                    nc.tensor.matmul(ps[:, 0:512], w1t[:],
                                     featsbf[:, bass.ts(s, 512)],
                                     start=True, stop=True)
                    nc.scalar.activation(xfull[:, bass.ts(s, 512)], ps[:, 0:512],
                                         AF.Prelu, bias=b1[:], scale=1.0, alpha=0.2)

                # table x|v columns: per 128-chunk transposed via matmul
                for q in range(8):
                    ps = zpps.tile([DM, 1024], f32, tag="zp")
                    for c in range(2):
                        ch = 2 * q + c
                        nc.tensor.matmul(ps[:, c * 512:c * 512 + DM],
                                         xfull[:, bass.ts(ch * 2, TQ)], identbf[:],
                                         start=True, stop=True)
                        nc.tensor.matmul(ps[:, c * 512 + DM:c * 512 + 2 * DM],
                                         xfull[:, bass.ts(ch * 2, TQ)], wvT[:],
                                         start=True, stop=True)
                        nc.tensor.matmul(ps[:, c * 512 + 2 * DM:c * 512 + 3 * DM],
                                         xfull[:, bass.ts(ch * 2 + 1, TQ)], identbf[:],
                                         start=True, stop=True)
                        nc.tensor.matmul(ps[:, c * 512 + 3 * DM:c * 512 + 4 * DM],
                                         xfull[:, bass.ts(ch * 2 + 1, TQ)], wvT[:],
                                         start=True, stop=True)
                    stg = spool.tile([TQ, 1024], f16, tag="stg")
                    nc.scalar.activation(stg[:], ps[:], AF.Copy)
                    # stg = [xT|vT|xT|vT] for 4 consecutive 128-row blocks;
                    # table row (q*512 + b*128 + p) <- stg[p, b-th 256-block]
                    nc.sync.dma_start(
                        out=table[q * 512:(q + 1) * 512, 0:2 * DM]
                        .rearrange("(b p) c -> p b c", b=4),
                        in_=stg[:].rearrange("p (b c) -> p b c", c=2 * DM))

                # xob = x for own queries
                xob = s1pool.tile([DM, NQ], bft)
                for s in range(4):
                    ps = zpps.tile([DM, 1024], f32, tag="zp")
                    nc.tensor.matmul(ps[:, 0:512], w1t[:], fob[:, bass.ts(s, 512)],
                                     start=True, stop=True)
                    nc.scalar.activation(xob[:, bass.ts(s, 512)], ps[:, 0:512],
                                         AF.Prelu, bias=b1[:], scale=1.0, alpha=0.2)

                # gqT / gpT blocks
                for g in range(4):
                    ps = scps.tile([TQ, 512], f32, tag="sc")
                    for c in range(4):
                        t = 4 * g + c
                        nc.tensor.matmul(ps[:, bass.ts(c, DM)],
                                         xob[:, bass.ts(t, TQ)], wq1t[:],
                                         start=True, stop=True)
                    nc.vector.tensor_copy(out=gqT[:, bass.ts(g, 512)], in_=ps[:])
                    ps = scps.tile([TQ, 512], f32, tag="sc")
                    for c in range(4):
                        t = 4 * g + c
                        nc.tensor.matmul(ps[:, bass.ts(c, DM)],
                                         poso[:, bass.ts(t, TQ)], wd1fT[:],
                                         start=True, stop=True)
                    nc.vector.tensor_copy(out=gpT[:, bass.ts(g, 512)], in_=ps[:])

            # ================= Phase B: per query tile =================
            with (
                tc.tile_pool(name="uarr", bufs=2) as upool,
                tc.tile_pool(name="topk", bufs=2) as kpool,
                tc.tile_pool(name="gath", bufs=2) as gpool,
                tc.tile_pool(name="pair", bufs=2) as prpool,
            ):
                for t in range(NT):
                    # ---- scores + pack ----
                    u = upool.tile([TQ, N], f32, tag="u")
                    cand = kpool.tile([TQ, 64], f32, tag="cand")
                    for h in range(4):
                        ps = scps.tile([TQ, 1024], f32, tag="sc")
                        for c in range(2):
                            nc.tensor.matmul(ps[:, bass.ts(c, 512)],
                                             lhsr[:, bass.ts(t, TQ)],
                                             rhsr[:, bass.ts(2 * h + c, 512)],
                                             start=True, stop=True)
                        nc.vector.scalar_tensor_tensor(
                            out=u[:, bass.ts(h, 1024)], in0=ps[:], scalar=BIAS,
                            in1=C[:, bass.ts(h, 1024)], op0=ALU.subtract, op1=ALU.add)
                        for c in range(2):
                            nc.vector.max(out=cand[:, bass.ts(2 * h + c, 8)],
                                          in_=u[:, bass.ts(2 * h + c, 512)])

                    # ---- merge to top16 ----
                    v16 = kpool.tile([TQ, 16], f32, tag="v16")
                    nc.vector.max(out=v16[:, 0:8], in_=cand[:])
                    repl = kpool.tile([TQ, 64], f32, tag="repl")
                    nc.vector.match_replace(out=repl[:], in_to_replace=v16[:, 0:8],
                                            in_values=cand[:], imm_value=-1e30)
                    nc.vector.max(out=v16[:, 8:16], in_=repl[:])

                    # ---- index extraction (Pool) ----
                    w16 = kpool.tile([TQ, 16], f32, tag="w16")
                    nc.vector.tensor_scalar(out=w16[:], in0=v16[:], scalar1=0.5,
                                            scalar2=None, op0=ALU.subtract)
                    S16 = kpool.tile([TQ, 16], f32, tag="S16")
                    nc.vector.tensor_scalar(out=S16[:], in0=w16[:], scalar1=CC,
                                            scalar2=CC, op0=ALU.add, op1=ALU.subtract)
                    g16 = kpool.tile([TQ, 16], f32, tag="g16")
                    nc.vector.tensor_sub(g16[:], v16[:], S16[:])
                    idxq = kpool.tile([TQ, TQ], f32, tag="idxq")
                    nc.vector.tensor_scalar(out=idxq[:, 0:16], in0=g16[:],
                                            scalar1=4096.0, scalar2=0.25,
                                            op0=ALU.mult, op1=ALU.subtract)
                    nc.vector.tensor_scalar(out=idxq[:, 16:32], in0=idxq[:, 0:16],
                                            scalar1=0.0, scalar2=4095.0,
                                            op0=ALU.max, op1=ALU.min)
                    nc.vector.tensor_copy(out=idxq[:, 0:16], in_=idxq[:, 16:32])
                    nc.vector.tensor_copy(out=idxq[:, 32:64], in_=idxq[:, 0:32])
                    nc.vector.tensor_copy(out=idxq[:, 64:128], in_=idxq[:, 0:64])
                    pt = scps.tile([TQ, 512], f32, tag="sc")
                    nc.tensor.transpose(pt[:, 0:TQ], idxq[:], identf[:])
                    idx16 = kpool.tile([TQ, TQ], i16, tag="idx16")
                    nc.vector.tensor_copy(out=idx16[:], in_=pt[:, 0:TQ])

                    # ---- gather [xT | vT | posaug] ----
                    gt = gpool.tile([DM, 12, 512], f16, tag="gt")
                    for g in range(4):
                        nc.gpsimd.dma_gather(
                            out_ap=gt[:, 3 * g:3 * g + 3, :], in_ap=table[:],
                            idxs_ap=idx16[:, bass.ts(g, 32)],
                            num_idxs=512, num_idxs_reg=512, elem_size=ROW,
                            transpose=True)

                    # ---- MLP halves ----
                    pe = prpool.tile([DM, PAIR], bft, tag="pe")
                    a2 = prpool.tile([DM, PAIR], bft, tag="a2")
                    ee = prpool.tile([DM, PAIR], bft, tag="ee")
                    for h in range(2):
                        hs = bass.ts(h, 1024)
                        zp = zpps.tile([DM, 1024], f32, tag="zp")
                        for c in range(2):
                            g = 2 * h + c
                            sl = bass.ts(c, 512)
                            nc.tensor.matmul(zp[:, sl], gpT[:, bass.ts(t, TQ)],
                                             emat[:, bass.ts(g, 512)],
                                             start=True, stop=False)
                            nc.tensor.matmul(zp[:, sl], wd1augT[:],
                                             gt[0:4, 3 * g + 2, :],
                                             start=False, stop=True)
                        h1 = prpool.tile([DM, 1024], bft, tag="h1")
                        nc.scalar.activation(h1[:], zp[:], AF.Prelu, bias=0.0,
                                             scale=1.0, alpha=0.2)
                        zp = zpps.tile([DM, 1024], f32, tag="zp")
                        for c in range(2):
                            sl = bass.ts(c, 512)
                            nc.tensor.matmul(zp[:, sl], wd2t[:], h1[:, sl],
                                             start=True, stop=True)
                        nc.scalar.activation(pe[:, hs], zp[:], AF.Prelu, bias=bd2[:],
                                             scale=1.0, alpha=0.2)
                        zp = zpps.tile([DM, 1024], f32, tag="zp")
                        for c in range(2):
                            g = 2 * h + c
                            sl = bass.ts(c, 512)
                            nc.tensor.matmul(zp[:, sl], gqT[:, bass.ts(t, TQ)],
                                             emat[:, bass.ts(g, 512)],
                                             start=True, stop=False)
                            nc.tensor.matmul(zp[:, sl], wg1kt[:],
                                             gt[:, 3 * g, :],
                                             start=False, stop=False)
                            nc.tensor.matmul(zp[:, sl], wg1t[:],
                                             pe[:, bass.ts(g, 512)],
                                             start=False, stop=True)
                        a1 = prpool.tile([DM, 1024], bft, tag="a1")
                        nc.scalar.activation(a1[:], zp[:], AF.Prelu, bias=bg1[:],
                                             scale=1.0, alpha=0.2)
                        zp = zpps.tile([DM, 1024], f32, tag="zp")
                        for c in range(2):
                            sl = bass.ts(c, 512)
                            nc.tensor.matmul(zp[:, sl], wg2t[:], a1[:, sl],
                                             start=True, stop=True)
                        nc.scalar.activation(a2[:, hs], zp[:], AF.Prelu, bias=bg2[:],
                                             scale=1.0, alpha=0.2)
                        nc.scalar.activation(ee[:, hs], a2[:, hs], AF.Exp,
                                             bias=0.0, scale=1.0 / 64.0)

                    # ---- softmax-normalized weighted sum ----
                    ww = prpool.tile([DM, PAIR], bft, tag="ww")
                    gtv = gt[:].rearrange("p (g r) n -> p g r n", r=3)
                    nc.vector.tensor_add(
                        ww[:].rearrange("p (g n) -> p g n", g=4),
                        gtv[:, :, 1, :],
                        pe[:].rearrange("p (g n) -> p g n", g=4))
                    uu = prpool.tile([DM, PAIR], bft, tag="uu")
                    nc.gpsimd.tensor_mul(uu[:], ee[:], ww[:])

                    # trees: L1 on DVE (2x), L2-4 on Pool
                    t1e = prpool.tile([DM, TQ, 8], bft, tag="t1e")
                    eev = ee[:].rearrange("p (q two k) -> p q two k", two=2, k=8)
                    nc.vector.tensor_add(t1e[:], eev[:, :, 0, :], eev[:, :, 1, :])
                    t1u = prpool.tile([DM, TQ, 8], bft, tag="t1u")
                    uuv = uu[:].rearrange("p (q two k) -> p q two k", two=2, k=8)
                    nc.vector.tensor_add(t1u[:], uuv[:, :, 0, :], uuv[:, :, 1, :])

                    t2e = prpool.tile([DM, TQ, 4], bft, tag="t2e")
                    t1ev = t1e[:].rearrange("p q (two k) -> p q two k", two=2)
                    nc.gpsimd.tensor_add(t2e[:], t1ev[:, :, 0, :], t1ev[:, :, 1, :])
                    t2u = prpool.tile([DM, TQ, 4], bft, tag="t2u")
                    t1uv = t1u[:].rearrange("p q (two k) -> p q two k", two=2)
                    nc.gpsimd.tensor_add(t2u[:], t1uv[:, :, 0, :], t1uv[:, :, 1, :])

                    t3e = prpool.tile([DM, TQ, 2], bft, tag="t3e")
                    t2ev = t2e[:].rearrange("p q (two k) -> p q two k", two=2)
                    nc.gpsimd.tensor_add(t3e[:], t2ev[:, :, 0, :], t2ev[:, :, 1, :])
                    t3u = prpool.tile([DM, TQ, 2], bft, tag="t3u")
                    t2uv = t2u[:].rearrange("p q (two k) -> p q two k", two=2)
                    nc.gpsimd.tensor_add(t3u[:], t2uv[:, :, 0, :], t2uv[:, :, 1, :])

                    ssum = kpool.tile([DM, TQ], f32, tag="ssum")
                    nc.gpsimd.tensor_add(ssum[:], t3e[:, :, 0], t3e[:, :, 1])
                    usum = kpool.tile([DM, TQ], f32, tag="usum")
                    nc.gpsimd.tensor_add(usum[:], t3u[:, :, 0], t3u[:, :, 1])

                    rrec = kpool.tile([DM, TQ], f32, tag="rrec")
                    nc.vector.reciprocal(rrec[:], ssum[:])
                    nc.gpsimd.tensor_mul(res_all[:, bass.ts(t, TQ)], usum[:], rrec[:])

            # ================= Phase C: output =================
            with tc.tile_pool(name="outp", bufs=2) as opool:
                o1 = opool.tile([DP, NQ], f32, tag="o1")
                for s in range(4):
                    ps = zpps.tile([DM, 1024], f32, tag="zp")
                    nc.tensor.matmul(ps[0:DP, 0:512], w2t[:],
                                     res_all[:, bass.ts(s, 512)],
                                     start=True, stop=True)
                    nc.scalar.activation(o1[:, bass.ts(s, 512)], ps[0:DP, 0:512],
                                         AF.Prelu, bias=b2[:], scale=1.0, alpha=0.2)
                o2 = opool.tile([DP, NQ], f32, tag="o2")
                nc.vector.tensor_add(o2[:], o1[:], fb_res[:])
                nc.sync.dma_start(out=out_d, in_=o2[:])

    nc.compile()
    return nc


def _host_prep(inputs):
    s1, b1 = _fold_bn(np.asarray(inputs["bn1"]))
    sd1, bd1 = _fold_bn(np.asarray(inputs["bnd1"]))
    sd2, bd2 = _fold_bn(np.asarray(inputs["bnd2"]))
    sg1, bg1 = _fold_bn(np.asarray(inputs["bng1"]))
    sg2, bg2 = _fold_bn(np.asarray(inputs["bng2"]))
    s2, b2 = _fold_bn(np.asarray(inputs["bn2"]))
    W1f = np.asarray(inputs["W1"]) * s1[:, None]
    Wd1f = np.asarray(inputs["Wd1"]) * sd1[:, None]
    Wd2f = np.asarray(inputs["Wd2"]) * sd2[:, None]
    Wg1f = np.asarray(inputs["Wg1"]) * sg1[:, None]
    Wg2f = np.asarray(inputs["Wg2"]) * sg2[:, None]
    W2f = np.asarray(inputs["W2"]) * s2[:, None]
    Wg1k = (Wg1f @ np.asarray(inputs["Wk"])).astype(np.float32)
    Wg1q = (Wg1f @ np.asarray(inputs["Wq"])).astype(np.float32)
    Wv = np.asarray(inputs["Wv"], np.float32)

    E = np.zeros((TQ, PAIR), np.float32)
    for q in range(TQ):
        E[q, q * K:(q + 1) * K] = 1.0

    wd1aug = np.zeros((4, DM), np.float32)
    wd1aug[0:3] = -Wd1f.T
    wd1aug[3] = bd1

    normL = np.zeros((65, 128), np.float32)
    normL[0:64] = -16.0
    normL[64] = 1.0

    iota = np.broadcast_to(
        ((np.arange(N) + 0.25) * 2.0 ** -12).astype(np.float32), (128, N)).copy()

    com = {
        "W1fT": np.ascontiguousarray(W1f.T, dtype=bf16),
        "b1": b1.reshape(DM, 1),
        "WvT": np.ascontiguousarray(Wv.T, dtype=bf16),
        "identbf": np.eye(DM, dtype=bf16),
        "identf": np.eye(TQ, dtype=np.float32),
        "Wd1fT": np.ascontiguousarray(Wd1f.T, dtype=bf16),
        "Wd1augT": wd1aug.astype(bf16),
        "Wd2fT": np.ascontiguousarray(Wd2f.T, dtype=bf16),
        "bd2": bd2.reshape(DM, 1),
        "Wg1qT": np.ascontiguousarray(Wg1q.T, dtype=bf16),
        "Wg1kTneg": np.ascontiguousarray(-Wg1k.T, dtype=bf16),
        "Wg1fT": np.ascontiguousarray(Wg1f.T, dtype=bf16),
        "bg1": bg1.reshape(DM, 1),
        "Wg2fT": np.ascontiguousarray(Wg2f.T, dtype=bf16),
        "bg2": bg2.reshape(DM, 1),
        "W2fT": np.ascontiguousarray(W2f.T, dtype=bf16),
        "b2": b2.reshape(DP, 1),
        "E": E.astype(bf16),
        "normL": normL,
        "iota_in": iota,
    }

    feats = np.asarray(inputs["feats"], np.float32)
    pos = np.asarray(inputs["pos"], np.float32)
    in_maps = []
    for c in range(8):
        b, half = c // 2, c % 2
        n0 = half * NQ
        fb = feats[b]
        lhsr = np.empty((65, NQ), np.float32)
        lhsr[0:64] = 32.0 * fb[:, n0:n0 + NQ]
        lhsr[64] = 1.0
        rhsr = np.empty((65, N), np.float32)
        rhsr[0:64] = fb
        rhsr[64] = BIAS
        posT = np.ones((N, 4), np.float16)
        posT[:, 0:3] = pos[b].T
        m = dict(com)
        m["lhsr"] = lhsr
        m["rhsr"] = rhsr
        m["fb_res"] = np.ascontiguousarray(fb[:, n0:n0 + NQ])
        m["feats32"] = np.ascontiguousarray(fb)
        m["featsbf"] = np.ascontiguousarray(fb, dtype=bf16)
        m["fob"] = np.ascontiguousarray(fb[:, n0:n0 + NQ], dtype=bf16)
        m["pos_own"] = np.ascontiguousarray(pos[b][:, n0:n0 + NQ], dtype=bf16)
        m["posT"] = posT
        in_maps.append(m)
    return in_maps


def kernel(**inputs):
    from concourse.bass_utils import run_bass_kernel_spmd

    if "nc" not in _CACHE:
        _CACHE["nc"] = _build_bass()
    nc = _CACHE["nc"]
    in_maps = _host_prep(inputs)
    r = run_bass_kernel_spmd(nc, in_maps, core_ids=list(range(8)),
                             **_CACHE.get("run_kwargs", {}))
    _CACHE["last_result"] = r
    out = np.empty((B, DP, N), np.float32)
    for c in range(8):
        b, half = c // 2, c % 2
        out[b][:, half * NQ:(half + 1) * NQ] = r.results[c]["out"]
    return out


# revision 8
# speedup vs baseline: 1.0460x; 1.0460x over previous
"""DGT block (dynamic graph transformer) Bass kernel for Trainium2 — v2.

Sharding: 8 cores = 4 batches x 2 query-halves (2048 queries/core vs all
4096 keys).

Key structure (vs v1):
  - kNN scores via ONE fp32r matmul per 512-key chunk with rows
    [32*f_n | bias-row B=2^24+2^22]. The PSUM write rounds (guard bit,
    single binade) to round(32*f.f) + B exactly.
  - One DVE scalar_tensor_tensor pass fuses PSUM eviction, de-bias, the
    norm subtraction and index packing:  u = (t - B) + C  where
    C = iota - round(16*||f||^2)  (device-built in setup).  u's integer
    part orders by quantized score (step 1/32), its fraction encodes the
    key index:  frac = (idx+0.25) * 2^-12.
  - top-16 = max8 per 512-window + match_replace merge; indices come from
    the fraction arithmetically (NO max_index full-row scans).
  - gather rows [xT | vT | pos,1] (768B); k/v projections fold into the
    attention matmuls post-gather (matmul cost is output-bound).
  - ww = v_g + pe, uu = ee*ww at bf16 2x; softmax reduces as tree adds
    (level 1 on DVE at 2x, levels 2-4 on Pool).
"""

import numpy as np
import ml_dtypes

B, N, K, DP, DM, EPS = 4, 4096, 16, 64, 128, 1e-5
NQ = N // 2            # queries per core
TQ = 128               # queries per tile
NT = NQ // TQ          # tiles per core (16)
PAIR = TQ * K          # pairs per tile (2048)
ROW = 3 * DM           # table row elems (f16): [xT(128) | vT(128) | pos4 | pad]
BIAS = float(2 ** 24 + 2 ** 22)
CC = float(2 ** 23 + 2 ** 22)

_CACHE = {}

bf16 = ml_dtypes.bfloat16


def _fold_bn(p):
    g, be, m, v = p.astype(np.float64)
    s = g / np.sqrt(v + EPS)
    return s.astype(np.float32), (be - m * s).astype(np.float32)


def _build_bass():
    import concourse.bass as bass
    import concourse.mybir as mybir
    import concourse.bacc as bacc
    from concourse.tile import TileContext

    dt = mybir.dt
    AF = mybir.ActivationFunctionType
    ALU = mybir.AluOpType

    nc = bacc.Bacc("TRN2", target_bir_lowering=False, debug=False, num_devices=8)

    def inp(name, shape, dtype):
        return nc.dram_tensor(name, list(shape), dtype, kind="ExternalInput").ap()

    lhsr_d = inp("lhsr", (65, NQ), dt.float32r)     # [32*f_own | ones]
    rhsr_d = inp("rhsr", (65, N), dt.float32r)      # [f_all | B-row]
    fb_res_d = inp("fb_res", (DP, NQ), dt.float32)  # residual
    feats32_d = inp("feats32", (DP, N), dt.float32)
    featsbf_d = inp("featsbf", (DP, N), dt.bfloat16)
    pos_own_d = inp("pos_own", (3, NQ), dt.bfloat16)
    fob_d = inp("fob", (DP, NQ), dt.bfloat16)
    posT_d = inp("posT", (N, 4), dt.float16)
    iota_d = inp("iota_in", (128, N), dt.float32)
    normL_d = inp("normL", (65, 128), dt.float32)   # [-16 x64 | 1]
    w1t_d = inp("W1fT", (DP, DM), dt.bfloat16)
    b1_d = inp("b1", (DM, 1), dt.float32)
    wvT_d = inp("WvT", (DM, DM), dt.bfloat16)
    identbf_d = inp("identbf", (DM, DM), dt.bfloat16)
    identf_d = inp("identf", (TQ, TQ), dt.float32)
    wd1fT_d = inp("Wd1fT", (3, DM), dt.bfloat16)
    wd1augT_d = inp("Wd1augT", (4, DM), dt.bfloat16)   # [-Wd1f^T ; bd1]
    wd2t_d = inp("Wd2fT", (DM, DM), dt.bfloat16)
    bd2_d = inp("bd2", (DM, 1), dt.float32)
    wq1t_d = inp("Wg1qT", (DM, DM), dt.bfloat16)
    wg1kt_d = inp("Wg1kTneg", (DM, DM), dt.bfloat16)   # -(Wg1f Wk)^T
    wg1t_d = inp("Wg1fT", (DM, DM), dt.bfloat16)
    bg1_d = inp("bg1", (DM, 1), dt.float32)
    wg2t_d = inp("Wg2fT", (DM, DM), dt.bfloat16)
    bg2_d = inp("bg2", (DM, 1), dt.float32)
    w2t_d = inp("W2fT", (DM, DP), dt.bfloat16)
    b2_d = inp("b2", (DP, 1), dt.float32)
    e_d = inp("E", (TQ, PAIR), dt.bfloat16)

    out_d = nc.dram_tensor("out", [DP, NQ], dt.float32, kind="ExternalOutput").ap()

    f32, f16, bft, i16 = dt.float32, dt.float16, dt.bfloat16, dt.int16

    with TileContext(nc) as tc:
        with (
            tc.tile_pool(name="const", bufs=1) as cpool,
            tc.tile_pool(name="persist", bufs=1) as ppool,
            tc.tile_pool(name="dram", bufs=1, space="DRAM") as dpool,
            tc.tile_pool(name="sc", bufs=2, space="PSUM") as scps,
            tc.tile_pool(name="zp", bufs=2, space="PSUM") as zpps,
        ):
            # ---- persistent constants ----
            lhsr = cpool.tile_from(lhsr_d)
            rhsr = cpool.tile_from(rhsr_d)
            w1t = cpool.tile_from(w1t_d)
            b1 = cpool.tile_from(b1_d)
            wvT = cpool.tile_from(wvT_d)
            identbf = cpool.tile_from(identbf_d)
            identf = cpool.tile_from(identf_d)
            wd1fT = cpool.tile_from(wd1fT_d)
            wd1augT = cpool.tile_from(wd1augT_d)
            wd2t = cpool.tile_from(wd2t_d)
            bd2 = cpool.tile_from(bd2_d)
            wq1t = cpool.tile_from(wq1t_d)
            wg1kt = cpool.tile_from(wg1kt_d)
            wg1t = cpool.tile_from(wg1t_d)
            bg1 = cpool.tile_from(bg1_d)
            wg2t = cpool.tile_from(wg2t_d)
            bg2 = cpool.tile_from(bg2_d)
            w2t = cpool.tile_from(w2t_d)
            b2 = cpool.tile_from(b2_d)
            emat = cpool.tile_from(e_d)
            normL = cpool.tile_from(normL_d)

            # ---- persistent working tensors ----
            C = ppool.tile([128, N], f32)          # iota - round(16||f||^2)
            xfull = ppool.tile([DM, N], bft)
            gqT = ppool.tile([TQ, NQ], bft)
            gpT = ppool.tile([TQ, NQ], bft)
            res_all = ppool.tile([DM, NQ], bft)
            fb_res = ppool.tile_from(fb_res_d)
            table = dpool.tile([N, ROW], f16)

            # ================= Phase A: setup =================
            with (
                tc.tile_pool(name="setup", bufs=2) as spool,
                tc.tile_pool(name="setup1", bufs=1) as s1pool,
            ):
                feats32 = s1pool.tile_from(feats32_d)
                featsbf = s1pool.tile_from(featsbf_d)
                poso = s1pool.tile_from(pos_own_d)
                fob = s1pool.tile_from(fob_d)
                iota = s1pool.tile_from(iota_d)

                # pos -> table cols [256:260]
                nc.sync.dma_start(out=table[:, 2 * DM:2 * DM + 4], in_=posT_d)

                # C = iota - round(16||f||^2): ffB rows [f*f | B]
                ffB = s1pool.tile([65, N], f32)
                nc.vector.memset(ffB[64:65, :], BIAS)
                for s in range(8):
                    nc.gpsimd.tensor_mul(ffB[0:64, bass.ts(s, 512)],
                                         feats32[:, bass.ts(s, 512)],
                                         feats32[:, bass.ts(s, 512)])
                for s in range(8):
                    ps = scps.tile([128, 512], f32, tag="sc")
                    nc.tensor.matmul(ps[:], normL[:], ffB[:, bass.ts(s, 512)],
                                     start=True, stop=True)
                    nc.vector.scalar_tensor_tensor(
                        out=C[:, bass.ts(s, 512)], in0=ps[:], scalar=BIAS,
                        in1=iota[:, bass.ts(s, 512)], op0=ALU.subtract, op1=ALU.add)

                # x = lrelu(bn1(W1 f)) for all keys
                for s in range(8):
                    ps = zpps.tile([DM, 1024], f32, tag="zp")
                    nc.tensor.matmul(ps[:, 0:512], w1t[:],
                                     featsbf[:, bass.ts(s, 512)],
                                     start=True, stop=True)
                    nc.scalar.activation(xfull[:, bass.ts(s, 512)], ps[:, 0:512],
                                         AF.Prelu, bias=b1[:], scale=1.0, alpha=0.2)

                # table x|v columns: per 128-chunk transposed via matmul
                for q in range(8):
                    ps = zpps.tile([DM, 1024], f32, tag="zp")
                    for c in range(2):
                        ch = 2 * q + c
                        nc.tensor.matmul(ps[:, c * 512:c * 512 + DM],
                                         xfull[:, bass.ts(ch * 2, TQ)], identbf[:],
                                         start=True, stop=True)
                        nc.tensor.matmul(ps[:, c * 512 + DM:c * 512 + 2 * DM],
                                         xfull[:, bass.ts(ch * 2, TQ)], wvT[:],
                                         start=True, stop=True)
                        nc.tensor.matmul(ps[:, c * 512 + 2 * DM:c * 512 + 3 * DM],
                                         xfull[:, bass.ts(ch * 2 + 1, TQ)], identbf[:],
                                         start=True, stop=True)
                        nc.tensor.matmul(ps[:, c * 512 + 3 * DM:c * 512 + 4 * DM],
                                         xfull[:, bass.ts(ch * 2 + 1, TQ)], wvT[:],
                                         start=True, stop=True)
                    stg = spool.tile([TQ, 1024], f16, tag="stg")
                    nc.scalar.activation(stg[:], ps[:], AF.Copy)
                    # stg = [xT|vT|xT|vT] for 4 consecutive 128-row blocks;
                    # table row (q*512 + b*128 + p) <- stg[p, b-th 256-block]
                    nc.sync.dma_start(
                        out=table[q * 512:(q + 1) * 512, 0:2 * DM]
                        .rearrange("(b p) c -> p b c", b=4),
                        in_=stg[:].rearrange("p (b c) -> p b c", c=2 * DM))

                # xob = x for own queries
                xob = s1pool.tile([DM, NQ], bft)
                for s in range(4):
                    ps = zpps.tile([DM, 1024], f32, tag="zp")
                    nc.tensor.matmul(ps[:, 0:512], w1t[:], fob[:, bass.ts(s, 512)],
                                     start=True, stop=True)
                    nc.scalar.activation(xob[:, bass.ts(s, 512)], ps[:, 0:512],
                                         AF.Prelu, bias=b1[:], scale=1.0, alpha=0.2)

                # gqT / gpT blocks
                for g in range(4):
                    ps = scps.tile([TQ, 512], f32, tag="sc")
                    for c in range(4):
                        t = 4 * g + c
                        nc.tensor.matmul(ps[:, bass.ts(c, DM)],
                                         xob[:, bass.ts(t, TQ)], wq1t[:],
                                         start=True, stop=True)
                    nc.vector.tensor_copy(out=gqT[:, bass.ts(g, 512)], in_=ps[:])
                    ps = scps.tile([TQ, 512], f32, tag="sc")
                    for c in range(4):
                        t = 4 * g + c
                        nc.tensor.matmul(ps[:, bass.ts(c, DM)],
                                         poso[:, bass.ts(t, TQ)], wd1fT[:],
                                         start=True, stop=True)
                    nc.vector.tensor_copy(out=gpT[:, bass.ts(g, 512)], in_=ps[:])

            # ================= Phase B: per query tile =================
            with (
                tc.tile_pool(name="uarr", bufs=2) as upool,
                tc.tile_pool(name="topk", bufs=2) as kpool,
                tc.tile_pool(name="gath", bufs=2) as gpool,
                tc.tile_pool(name="pair", bufs=2) as prpool,
            ):
                def emit_topk(t):
                    # ---- scores + pack ----
                    u = upool.tile([TQ, N], f32, tag="u")
                    cand = kpool.tile([TQ, 64], f32, tag="cand")
                    for h in range(4):
                        ps = scps.tile([TQ, 1024], f32, tag="sc")
                        for c in range(2):
                            nc.tensor.matmul(ps[:, bass.ts(c, 512)],
                                             lhsr[:, bass.ts(t, TQ)],
                                             rhsr[:, bass.ts(2 * h + c, 512)],
                                             start=True, stop=True)
                        nc.vector.scalar_tensor_tensor(
                            out=u[:, bass.ts(h, 1024)], in0=ps[:], scalar=BIAS,
                            in1=C[:, bass.ts(h, 1024)], op0=ALU.subtract, op1=ALU.add)
                        for c in range(2):
                            nc.vector.max(out=cand[:, bass.ts(2 * h + c, 8)],
                                          in_=u[:, bass.ts(2 * h + c, 512)])

                    # ---- merge to top16 ----
                    v16 = kpool.tile([TQ, 16], f32, tag="v16")
                    nc.vector.max(out=v16[:, 0:8], in_=cand[:])
                    repl = kpool.tile([TQ, 64], f32, tag="repl")
                    nc.vector.match_replace(out=repl[:], in_to_replace=v16[:, 0:8],
                                            in_values=cand[:], imm_value=-1e30)
                    nc.vector.max(out=v16[:, 8:16], in_=repl[:])

                    # ---- index extraction ----
                    w16 = kpool.tile([TQ, 16], f32, tag="w16")
                    nc.vector.tensor_scalar(out=w16[:], in0=v16[:], scalar1=0.5,
                                            scalar2=None, op0=ALU.subtract)
                    S16 = kpool.tile([TQ, 16], f32, tag="S16")
                    nc.vector.tensor_scalar(out=S16[:], in0=w16[:], scalar1=CC,
                                            scalar2=CC, op0=ALU.add, op1=ALU.subtract)
                    g16 = kpool.tile([TQ, 16], f32, tag="g16")
                    nc.vector.tensor_sub(g16[:], v16[:], S16[:])
                    idxq = kpool.tile([TQ, TQ], f32, tag="idxq")
                    nc.vector.tensor_scalar(out=idxq[:, 0:16], in0=g16[:],
                                            scalar1=4096.0, scalar2=0.25,
                                            op0=ALU.mult, op1=ALU.subtract)
                    nc.vector.tensor_scalar(out=idxq[:, 16:32], in0=idxq[:, 0:16],
                                            scalar1=0.0, scalar2=4095.0,
                                            op0=ALU.max, op1=ALU.min)
                    nc.vector.tensor_copy(out=idxq[:, 0:16], in_=idxq[:, 16:32])
                    nc.vector.tensor_copy(out=idxq[:, 32:64], in_=idxq[:, 0:32])
                    nc.vector.tensor_copy(out=idxq[:, 64:128], in_=idxq[:, 0:64])
                    pt = scps.tile([TQ, 512], f32, tag="sc")
                    nc.tensor.transpose(pt[:, 0:TQ], idxq[:], identf[:])
                    idx16 = kpool.tile([TQ, TQ], i16, tag="idx16")
                    nc.vector.tensor_copy(out=idx16[:], in_=pt[:, 0:TQ])

                    # ---- gather [xT | vT | posaug] ----
                    gt = gpool.tile([DM, 12, 512], f16, tag="gt")
                    for g in range(4):
                        nc.gpsimd.dma_gather(
                            out_ap=gt[:, 3 * g:3 * g + 3, :], in_ap=table[:],
                            idxs_ap=idx16[:, bass.ts(g, 32)],
                            num_idxs=512, num_idxs_reg=512, elem_size=ROW,
                            transpose=True)
                    return gt

                def emit_mlp(t, gt):
                    pe = prpool.tile([DM, PAIR], bft, tag="pe")
                    a2 = prpool.tile([DM, PAIR], bft, tag="a2")
                    ee = prpool.tile([DM, PAIR], bft, tag="ee")
                    for h in range(2):
                        hs = bass.ts(h, 1024)
                        zp = zpps.tile([DM, 1024], f32, tag="zp")
                        for c in range(2):
                            g = 2 * h + c
                            sl = bass.ts(c, 512)
                            nc.tensor.matmul(zp[:, sl], gpT[:, bass.ts(t, TQ)],
                                             emat[:, bass.ts(g, 512)],
                                             start=True, stop=False)
                            nc.tensor.matmul(zp[:, sl], wd1augT[:],
                                             gt[0:4, 3 * g + 2, :],
                                             start=False, stop=True)
                        h1 = prpool.tile([DM, 1024], bft, tag="h1")
                        nc.scalar.activation(h1[:], zp[:], AF.Prelu, bias=0.0,
                                             scale=1.0, alpha=0.2)
                        zp = zpps.tile([DM, 1024], f32, tag="zp")
                        for c in range(2):
                            sl = bass.ts(c, 512)
                            nc.tensor.matmul(zp[:, sl], wd2t[:], h1[:, sl],
                                             start=True, stop=True)
                        nc.scalar.activation(pe[:, hs], zp[:], AF.Prelu, bias=bd2[:],
                                             scale=1.0, alpha=0.2)
                        zp = zpps.tile([DM, 1024], f32, tag="zp")
                        for c in range(2):
                            g = 2 * h + c
                            sl = bass.ts(c, 512)
                            nc.tensor.matmul(zp[:, sl], gqT[:, bass.ts(t, TQ)],
                                             emat[:, bass.ts(g, 512)],
                                             start=True, stop=False)
                            nc.tensor.matmul(zp[:, sl], wg1kt[:],
                                             gt[:, 3 * g, :],
                                             start=False, stop=False)
                            nc.tensor.matmul(zp[:, sl], wg1t[:],
                                             pe[:, bass.ts(g, 512)],
                                             start=False, stop=True)
                        a1 = prpool.tile([DM, 1024], bft, tag="a1")
                        nc.scalar.activation(a1[:], zp[:], AF.Prelu, bias=bg1[:],
                                             scale=1.0, alpha=0.2)
                        zp = zpps.tile([DM, 1024], f32, tag="zp")
                        for c in range(2):
                            sl = bass.ts(c, 512)
                            nc.tensor.matmul(zp[:, sl], wg2t[:], a1[:, sl],
                                             start=True, stop=True)
                        nc.scalar.activation(a2[:, hs], zp[:], AF.Prelu, bias=bg2[:],
                                             scale=1.0, alpha=0.2)
                        nc.scalar.activation(ee[:, hs], a2[:, hs], AF.Exp,
                                             bias=0.0, scale=1.0 / 64.0)

                    # ---- softmax-normalized weighted sum ----
                    ww = prpool.tile([DM, PAIR], bft, tag="ww")
                    gtv = gt[:].rearrange("p (g r) n -> p g r n", r=3)
                    nc.vector.tensor_add(
                        ww[:].rearrange("p (g n) -> p g n", g=4),
                        gtv[:, :, 1, :],
                        pe[:].rearrange("p (g n) -> p g n", g=4))
                    uu = prpool.tile([DM, PAIR], bft, tag="uu")
                    nc.gpsimd.tensor_mul(uu[:], ee[:], ww[:])

                    t1e = prpool.tile([DM, TQ, 8], bft, tag="t1e")
                    eev = ee[:].rearrange("p (q two k) -> p q two k", two=2, k=8)
                    nc.vector.tensor_add(t1e[:], eev[:, :, 0, :], eev[:, :, 1, :])
                    t1u = prpool.tile([DM, TQ, 8], bft, tag="t1u")
                    uuv = uu[:].rearrange("p (q two k) -> p q two k", two=2, k=8)
                    nc.vector.tensor_add(t1u[:], uuv[:, :, 0, :], uuv[:, :, 1, :])

                    t2e = prpool.tile([DM, TQ, 4], bft, tag="t2e")
                    t1ev = t1e[:].rearrange("p q (two k) -> p q two k", two=2)
                    nc.gpsimd.tensor_add(t2e[:], t1ev[:, :, 0, :], t1ev[:, :, 1, :])
                    t2u = prpool.tile([DM, TQ, 4], bft, tag="t2u")
                    t1uv = t1u[:].rearrange("p q (two k) -> p q two k", two=2)
                    nc.gpsimd.tensor_add(t2u[:], t1uv[:, :, 0, :], t1uv[:, :, 1, :])

                    t3e = prpool.tile([DM, TQ, 2], bft, tag="t3e")
                    t2ev = t2e[:].rearrange("p q (two k) -> p q two k", two=2)
                    nc.gpsimd.tensor_add(t3e[:], t2ev[:, :, 0, :], t2ev[:, :, 1, :])
                    t3u = prpool.tile([DM, TQ, 2], bft, tag="t3u")
                    t2uv = t2u[:].rearrange("p q (two k) -> p q two k", two=2)
                    nc.gpsimd.tensor_add(t3u[:], t2uv[:, :, 0, :], t2uv[:, :, 1, :])

                    ssum = kpool.tile([DM, TQ], f32, tag="ssum")
                    nc.gpsimd.tensor_add(ssum[:], t3e[:, :, 0], t3e[:, :, 1])
                    usum = kpool.tile([DM, TQ], f32, tag="usum")
                    nc.gpsimd.tensor_add(usum[:], t3u[:, :, 0], t3u[:, :, 1])

                    rrec = kpool.tile([DM, TQ], f32, tag="rrec")
                    nc.vector.reciprocal(rrec[:], ssum[:])
                    nc.gpsimd.tensor_mul(res_all[:, bass.ts(t, TQ)], usum[:], rrec[:])

                gt_prev = None
                for t in range(NT + 1):
                    if t < NT:
                        gt_new = emit_topk(t)
                    if t >= 1:
                        emit_mlp(t - 1, gt_prev)
                    if t < NT:
                        gt_prev = gt_new

            # ================= Phase C: output =================
            with tc.tile_pool(name="outp", bufs=2) as opool:
                o1 = opool.tile([DP, NQ], f32, tag="o1")
                for s in range(4):
                    ps = zpps.tile([DM, 1024], f32, tag="zp")
                    nc.tensor.matmul(ps[0:DP, 0:512], w2t[:],
                                     res_all[:, bass.ts(s, 512)],
                                     start=True, stop=True)
                    nc.scalar.activation(o1[:, bass.ts(s, 512)], ps[0:DP, 0:512],
                                         AF.Prelu, bias=b2[:], scale=1.0, alpha=0.2)
                o2 = opool.tile([DP, NQ], f32, tag="o2")
                nc.vector.tensor_add(o2[:], o1[:], fb_res[:])
                nc.sync.dma_start(out=out_d, in_=o2[:])

    nc.compile()
    return nc


def _host_prep(inputs):
    s1, b1 = _fold_bn(np.asarray(inputs["bn1"]))
    sd1, bd1 = _fold_bn(np.asarray(inputs["bnd1"]))
    sd2, bd2 = _fold_bn(np.asarray(inputs["bnd2"]))
    sg1, bg1 = _fold_bn(np.asarray(inputs["bng1"]))
    sg2, bg2 = _fold_bn(np.asarray(inputs["bng2"]))
    s2, b2 = _fold_bn(np.asarray(inputs["bn2"]))
    W1f = np.asarray(inputs["W1"]) * s1[:, None]
    Wd1f = np.asarray(inputs["Wd1"]) * sd1[:, None]
    Wd2f = np.asarray(inputs["Wd2"]) * sd2[:, None]
    Wg1f = np.asarray(inputs["Wg1"]) * sg1[:, None]
    Wg2f = np.asarray(inputs["Wg2"]) * sg2[:, None]
    W2f = np.asarray(inputs["W2"]) * s2[:, None]
    Wg1k = (Wg1f @ np.asarray(inputs["Wk"])).astype(np.float32)
    Wg1q = (Wg1f @ np.asarray(inputs["Wq"])).astype(np.float32)
    Wv = np.asarray(inputs["Wv"], np.float32)

    E = np.zeros((TQ, PAIR), np.float32)
    for q in range(TQ):
        E[q, q * K:(q + 1) * K] = 1.0

    wd1aug = np.zeros((4, DM), np.float32)
    wd1aug[0:3] = -Wd1f.T
    wd1aug[3] = bd1

    normL = np.zeros((65, 128), np.float32)
    normL[0:64] = -16.0
    normL[64] = 1.0

    iota = np.broadcast_to(
        ((np.arange(N) + 0.25) * 2.0 ** -12).astype(np.float32), (128, N)).copy()

    com = {
        "W1fT": np.ascontiguousarray(W1f.T, dtype=bf16),
        "b1": b1.reshape(DM, 1),
        "WvT": np.ascontiguousarray(Wv.T, dtype=bf16),
        "identbf": np.eye(DM, dtype=bf16),
        "identf": np.eye(TQ, dtype=np.float32),
        "Wd1fT": np.ascontiguousarray(Wd1f.T, dtype=bf16),
        "Wd1augT": wd1aug.astype(bf16),
        "Wd2fT": np.ascontiguousarray(Wd2f.T, dtype=bf16),
        "bd2": bd2.reshape(DM, 1),
        "Wg1qT": np.ascontiguousarray(Wg1q.T, dtype=bf16),
        "Wg1kTneg": np.ascontiguousarray(-Wg1k.T, dtype=bf16),
        "Wg1fT": np.ascontiguousarray(Wg1f.T, dtype=bf16),
        "bg1": bg1.reshape(DM, 1),
        "Wg2fT": np.ascontiguousarray(Wg2f.T, dtype=bf16),
        "bg2": bg2.reshape(DM, 1),
        "W2fT": np.ascontiguousarray(W2f.T, dtype=bf16),
        "b2": b2.reshape(DP, 1),
        "E": E.astype(bf16),
        "normL": normL,
        "iota_in": iota,
    }

    feats = np.asarray(inputs["feats"], np.float32)
    pos = np.asarray(inputs["pos"], np.float32)
    in_maps = []
    for c in range(8):
        b, half = c // 2, c % 2
        n0 = half * NQ
        fb = feats[b]
        lhsr = np.empty((65, NQ), np.float32)
        lhsr[0:64] = 32.0 * fb[:, n0:n0 + NQ]
        lhsr[64] = 1.0
        rhsr = np.empty((65, N), np.float32)
        rhsr[0:64] = fb
        rhsr[64] = BIAS
        posT = np.ones((N, 4), np.float16)
        posT[:, 0:3] = pos[b].T
        m = dict(com)
        m["lhsr"] = lhsr
        m["rhsr"] = rhsr
        m["fb_res"] = np.ascontiguousarray(fb[:, n0:n0 + NQ])
        m["feats32"] = np.ascontiguousarray(fb)
        m["featsbf"] = np.ascontiguousarray(fb, dtype=bf16)
        m["fob"] = np.ascontiguousarray(fb[:, n0:n0 + NQ], dtype=bf16)
        m["pos_own"] = np.ascontiguousarray(pos[b][:, n0:n0 + NQ], dtype=bf16)
        m["posT"] = posT
        in_maps.append(m)
    return in_maps


def kernel(**inputs):
    from concourse.bass_utils import run_bass_kernel_spmd

    if "nc" not in _CACHE:
        _CACHE["nc"] = _build_bass()
    nc = _CACHE["nc"]
    in_maps = _host_prep(inputs)
    r = run_bass_kernel_spmd(nc, in_maps, core_ids=list(range(8)),
                             **_CACHE.get("run_kwargs", {}))
    _CACHE["last_result"] = r
    out = np.empty((B, DP, N), np.float32)
    for c in range(8):
        b, half = c // 2, c % 2
        out[b][:, half * NQ:(half + 1) * NQ] = r.results[c]["out"]
    return out
